# revision 31
# baseline (speedup 1.0000x reference)
"""Trainium2 Bass kernel for nn_Block_627065225827 (dense_transformer).

Self-contained: hardcodes shapes B=32, T=4096, C=256, H=8 and the
data-parallel-over-batch sharding (4 batch rows per core, 8 cores).

Math (see reference):
    h   = LN1(x) * g1 + b1ln
    id  = h @ w_id.T ;  inf = h @ w_inf.T            (per-head view [H, hs])
    inf = inf / (1+K);  shifted[t] = inf[t - s_h]    (zero for t < s_h)
    sa  = (K/(1+K) * id + shifted) @ w_proj.T + b_proj
    x1  = x + sa
    ff  = relu(LN2(x1)*g2+b2ln @ w1.T + b1) @ w2.T + b2
    out = x1 + ff

Two-stage attention (rank-structure, exact):
    y[c,t] = (a . w_id_g h)[c,t] + (b . w_inf_g h)[c, t - s_c]
    sa     = w_proj_perm @ y
  with all 256 y-channels permuted so shift groups are contiguous
  (band of 64 per shift s=1..4). The per-band temporal shift is applied
  by accumulating each band's matmul into PSUM with a shifted rhs token
  window read from hB (channel-major h-hat, 16 zero pad columns at
  head). id (and the base-partition-0 inf bands) run fp8 DoubleRow;
  offset-64 bands run normal-mode fp8 (DR requires dst partition 0).
  FFN up runs fp8 DoubleRow with paired psum tiles ([128,1024] across
  two banks) and one relu per pair. Weights are pre-scaled (x64 attn,
  x16 up), compensated in the psum->sbuf copy / relu input scale.
  x and x1 are kept in bf16 (cast on DMA load); the residual-path
  quantization (~0.4% of |out|) fits the error budget.

Schedule: 2 batch rows interleaved; x loads + LN1 stats prefetched two
windows ahead; LN1-apply of w+1 and the down-proj of w-1 are emitted
as fillers between the up-proj pairs of w to keep the PE stream dense
(HAM warm) and spread V/S queue pressure.
"""

import os
from contextlib import ExitStack

import numpy as np
import ml_dtypes

B, T, C, H = 32, 4096, 256, 8
HS = C // H
NCORES = 8
BPC = B // NCORES  # batch rows per core
SHIFTS = [1, 2, 3, 4, 1, 2, 3, 4]
EPS = 1e-5
PADW = 16  # zero columns at the head of hB (keeps DoubleRow plane step %16)
WIN = 512  # tokens per window
SUB = 128  # tokens per subtile (partition dim)

_f64 = np.float64
_bf16 = ml_dtypes.bfloat16
_fp8 = ml_dtypes.float8_e4m3

FP8_ATT = True   # stage1 (id/inf) in fp8 (id + base-0 bands DoubleRow)
FP8_UP = True    # FFN up in fp8 DoubleRow
ATT_W_SCALE = 64.0
UP_W_SCALE = 16.0
RELU_PAIR = True  # one relu per up-pair ([128,1024] across 2 psum banks)

# engine for flexible psum->sbuf copies: 'v' | 's'
EG = {
    "hb_copy": "s",  # merged transpose psum -> hB (fp8 cast), 2/window
    "h2_copy": "s",  # merged transpose psum -> h2 (fp8 cast), 2/window
    "y_copy": "s",   # stage1 psum -> yB bf16 (scale 1/ATT_W_SCALE)
}
HN_S = 0  # how many of the 4 per-window-slot hn tensor_scalars run on scalar

# permutation of the 256 y channels: group by shift (s=1..4, 64 ch each)
PERM = np.concatenate(
    [np.concatenate([np.arange(h * HS, (h + 1) * HS) for h in range(H)
                     if SHIFTS[h] == s]) for s in (1, 2, 3, 4)]
)


def _prep(inputs):
    """Fold LN gains/biases + per-head scalars into the weights (host, numpy)."""
    g = {k: np.asarray(v, dtype=_f64) for k, v in inputs.items() if k != "x"}
    K = np.exp(g["khead"])  # [H]
    a_row = np.repeat(K / (1.0 + K), HS)  # [C] per id-output channel
    b_row = np.repeat(1.0 / (1.0 + K), HS)  # [C] per inf-output channel

    w_id_g = g["w_id"] * g["ln1_g"][None, :]
    w_inf_g = g["w_inf"] * g["ln1_g"][None, :]
    w_id_s = (w_id_g * a_row[:, None])[PERM, :]   # permuted rows
    w_inf_s = (w_inf_g * b_row[:, None])[PERM, :]
    w_proj_p = g["w_proj"][:, PERM]

    # LN bias / proj / ffn bias constants must be zero for this kernel
    # structure (true for this problem instance).
    assert np.abs(g["ln1_b"]).max() == 0 and np.abs(g["b_proj"]).max() == 0
    assert np.abs(g["ln2_b"]).max() == 0 and np.abs(g["b1"]).max() == 0
    assert np.abs(g["b2"]).max() == 0

    att_dt = _fp8 if FP8_ATT else _bf16
    att_s = ATT_W_SCALE if FP8_ATT else 1.0

    def dr_pack(wT):  # [C_in, M] -> [128, 2, M] (contraction = 128*j + k)
        Min = wT.shape[1]
        return np.ascontiguousarray(wT.reshape(2, 128, Min).transpose(1, 0, 2))

    w1_g = g["w1"] * g["ln2_g"][None, :]
    w1dr = dr_pack(w1_g.T * UP_W_SCALE)  # [128, 2, 4C]

    out = {
        "widT": dr_pack(w_id_s.T * att_s).astype(att_dt),
        "winfT": dr_pack(w_inf_s.T * att_s).astype(att_dt),
        "wprojT": np.ascontiguousarray(w_proj_p.T.reshape(2, 128, C)).astype(_bf16),
        "w1dr": np.ascontiguousarray(w1dr).astype(_fp8 if FP8_UP else _bf16),
        "w2T": np.ascontiguousarray(g["w2"].T.reshape(8, 128, C)).astype(_bf16),
    }
    return out


def _build(n_rows=BPC, t_len=T):
    """Build + compile the per-core Bass program. Returns the finalized nc."""
    import concourse.bacc as bacc
    import concourse.bass as bass
    import concourse.mybir as mybir
    import concourse.tile as tile
    from concourse.masks import make_identity

    dt = mybir.dt
    AF = mybir.ActivationFunctionType
    OP = mybir.AluOpType
    DR = mybir.MatmulPerfMode.DoubleRow

    att_dt = dt.float8e4 if FP8_ATT else dt.bfloat16
    up_dt = dt.float8e4 if FP8_UP else dt.bfloat16

    nwin = t_len // WIN
    nc = bacc.Bacc("TRN2", target_bir_lowering=False, debug=False, num_devices=NCORES)

    x_d = nc.declare_dram_parameter("x", [n_rows, t_len, C], dt.float32, isOutput=False)
    widT_d = nc.declare_dram_parameter("widT", [128, 2, C], att_dt, isOutput=False)
    winfT_d = nc.declare_dram_parameter("winfT", [128, 2, C], att_dt, isOutput=False)
    wprojT_d = nc.declare_dram_parameter("wprojT", [2, 128, C], dt.bfloat16, isOutput=False)
    w1dr_d = nc.declare_dram_parameter("w1dr", [128, 2, 4 * C], up_dt, isOutput=False)
    w2T_d = nc.declare_dram_parameter("w2T", [8, 128, C], dt.bfloat16, isOutput=False)
    out_d = nc.declare_dram_parameter("out", [n_rows, t_len, C], dt.float32, isOutput=True)

    with tile.TileContext(nc) as tc, ExitStack() as ctx:
        singles = ctx.enter_context(tc.tile_pool(name="singles", bufs=1))
        hb_pool = ctx.enter_context(tc.tile_pool(name="hb", bufs=2))
        xin = ctx.enter_context(tc.tile_pool(name="xin", bufs=8))
        x1p = ctx.enter_context(tc.tile_pool(name="x1p", bufs=16))
        hnorm = ctx.enter_context(tc.tile_pool(name="hnorm", bufs=12))
        stats = ctx.enter_context(tc.tile_pool(name="stats", bufs=24))
        ybp = ctx.enter_context(tc.tile_pool(name="ybp", bufs=3))
        h2b = ctx.enter_context(tc.tile_pool(name="h2b", bufs=3))
        ffb = ctx.enter_context(tc.tile_pool(name="ffb", bufs=2))
        outp = ctx.enter_context(tc.tile_pool(name="outp", bufs=10))
        tp_ps = ctx.enter_context(tc.tile_pool(name="tp_ps", bufs=1, space="PSUM"))
        acc_ps = ctx.enter_context(tc.tile_pool(name="acc_ps", bufs=2, space="PSUM"))
        y_ps = ctx.enter_context(tc.tile_pool(name="y_ps", bufs=1, space="PSUM"))
        up_ps = ctx.enter_context(tc.tile_pool(name="up_ps", bufs=2, space="PSUM"))

        # ---- constants / weights in SBUF ----
        ident = singles.tile([128, 128], dt.bfloat16)
        make_identity(nc, ident)
        eps_t = singles.tile([128, 1], dt.float32)
        nc.vector.memset(eps_t, EPS)
        widT = singles.tile([128, 2, C], att_dt, tag="widT")
        nc.sync.dma_start(out=widT, in_=widT_d[:, :, :])
        winfT = singles.tile([128, 2, C], att_dt, tag="winfT")
        nc.sync.dma_start(out=winfT, in_=winfT_d[:, :, :])
        wprojT = []
        for kc in range(2):
            w = singles.tile([128, C], dt.bfloat16, tag=f"wprojT{kc}")
            nc.sync.dma_start(out=w, in_=wprojT_d[kc])
            wprojT.append(w)
        w1dr = singles.tile([128, 2, 4 * C], up_dt, tag="w1dr")
        nc.sync.dma_start(out=w1dr, in_=w1dr_d[:, :, :])
        w2T = []
        for fc in range(8):
            w = singles.tile([128, C], dt.bfloat16, tag=f"w2T{fc}")
            nc.sync.dma_start(out=w, in_=w2T_d[fc])
            w2T.append(w)

        def ln_stats(src_tile, mvW, k):
            st = stats.tile([128, 6], dt.float32, tag="st", name="st")
            nc.vector.bn_stats(st, src_tile)
            nc.vector.bn_aggr(mvW[:, 2 * k:2 * k + 2], st)

        def ln_batch_rsqrt(mvW):
            sdW = stats.tile([128, 4], dt.float32, tag="sd", name="sd")
            var_view = bass.AP(tensor=mvW.tensor, offset=mvW.offset + 1,
                               ap=[mvW.ap[0], [2, 4]])
            nc.scalar.activation(sdW, var_view, AF.Sqrt, bias=eps_t, scale=1.0)
            rsW = stats.tile([128, 4], dt.float32, tag="rs", name="rs")
            nc.vector.reciprocal(rsW, sdW)
            return rsW

        def ln_pair(src_tiles, mvW, rsW, kp, dst_tile, dst_col, dst_cstride,
                    copy_eng):
            """LN-apply + transpose two token subtiles (k = 2*kp, 2*kp+1) and
            copy the merged [128, 512] psum into the channel-major dst."""
            pt = tp_ps.tile([128, 4 * SUB], dt.float32, tag="tp", name="tp")
            for kk in range(2):
                k = 2 * kp + kk
                hn = hnorm.tile([128, C], dt.bfloat16, tag="hn", name="hn")
                nc.vector.tensor_scalar(
                    out=hn, in0=src_tiles[k], scalar1=mvW[:, 2 * k:2 * k + 1],
                    scalar2=rsW[:, k:k + 1], op0=OP.subtract, op1=OP.mult,
                )
                for c in range(2):
                    nc.tensor.matmul(
                        pt[:, (2 * kk + c) * SUB:(2 * kk + c + 1) * SUB],
                        hn[:, 128 * c:128 * (c + 1)], ident,
                        start=True, stop=True)
            src4 = bass.AP(tensor=pt.tensor, offset=pt.offset,
                           ap=[pt.ap[0], [SUB, 2], [2 * SUB, 2], [1, SUB]])
            dst4 = bass.AP(tensor=dst_tile.tensor,
                           offset=dst_tile.offset + dst_col,
                           ap=[dst_tile.ap[0], [dst_cstride, 2], [SUB, 2], [1, SUB]])
            if copy_eng == "s":
                nc.scalar.copy(out=dst4, in_=src4)
            else:
                nc.vector.tensor_copy(out=dst4, in_=src4)

        state = {}

        def x_load(slot, r, w):
            """One DMA (f32->bf16 cast) for window w's 4 x subtiles + LN1 stats.
            SBUF layout [128, 4, C]: token t0w + 128*k + p at [p, k, :]."""
            t0w = w * WIN
            xw = xin.tile([128, 4, C], dt.bfloat16, tag="x", name="x")
            src = x_d[r, t0w:t0w + WIN, :].rearrange("(k p) c -> p k c", p=128)
            nc.gpsimd.dma_start(out=xw, in_=src)
            x_tiles = [xw[:, k, :] for k in range(4)]
            mv1 = stats.tile([128, 8], dt.float32, tag="mv1", name="mv1")
            for k in range(4):
                ln_stats(x_tiles[k], mv1, k)
            state[("x", slot, w)] = x_tiles
            state[("mv1", slot, w)] = mv1

        def ln1_piece(slot, hB, w, kp):
            """LN1-apply + transpose of k-pair kp of window w into hB."""
            if kp == 0:
                state[("rs1", slot, w)] = ln_batch_rsqrt(state[("mv1", slot, w)])
            ln_pair(state[("x", slot, w)], state[("mv1", slot, w)],
                    state[("rs1", slot, w)], kp, hB,
                    PADW + w * WIN + kp * 2 * SUB, PADW + t_len, EG["hb_copy"])

        def stage1_phase(slot, hB, r, w):
            col0 = PADW + w * WIN
            yb = ybp.tile([128, 2, WIN], dt.bfloat16, tag=f"yb{slot}",
                          name=f"yb{slot}")
            state[("yb", slot)] = yb
            for mc in range(2):
                yp = y_ps.tile([128, WIN], dt.float32, tag="y", name="y")
                if FP8_ATT:
                    # id: full 128-partition dst -> DoubleRow ok.
                    nc.tensor.matmul(yp, widT[:, :, 128 * mc:128 * (mc + 1)],
                                     hB[:, :, col0:col0 + WIN],
                                     start=True, stop=False, perf_mode=DR)
                    # band 0 dst is partition base 0 -> DoubleRow ok
                    s = 2 * mc + 1
                    nc.tensor.matmul(
                        yp[0:64, :], winfT[:, :, 128 * mc:128 * mc + 64],
                        hB[:, :, col0 - s:col0 - s + WIN],
                        start=False, stop=False, perf_mode=DR)
                    # band 1 dst at partition 64: DR invalid -> normal fp8
                    s = 2 * mc + 2
                    for kc in range(2):
                        nc.tensor.matmul(
                            yp[64:128, :],
                            winfT[:, kc, 128 * mc + 64:128 * mc + 128],
                            hB[:, kc, col0 - s:col0 - s + WIN],
                            start=False, stop=(kc == 1))
                else:
                    for kc in range(2):
                        nc.tensor.matmul(yp, widT[:, kc, 128 * mc:128 * (mc + 1)],
                                         hB[:, kc, col0:col0 + WIN],
                                         start=(kc == 0), stop=False)
                    for b in range(2):
                        s = 2 * mc + b + 1
                        for kc in range(2):
                            nc.tensor.matmul(
                                yp[64 * b:64 * (b + 1), :],
                                winfT[:, kc, 128 * mc + 64 * b:128 * mc + 64 * (b + 1)],
                                hB[:, kc, col0 - s:col0 - s + WIN],
                                start=False, stop=(b == 1 and kc == 1))
                yscale = 1.0 / ATT_W_SCALE if FP8_ATT else 1.0
                nc.scalar.mul(yb[:, mc, :], yp, yscale)

        def stage2_phase(slot, r, w):
            x_tiles = state[("x", slot, w)]
            yb = state[("yb", slot)]
            x1_tiles = []
            mv2 = stats.tile([128, 8], dt.float32, tag="mv2", name="mv2")
            for k in range(4):
                ps = acc_ps.tile([128, C], dt.float32, tag="acc", name="sa")
                for kc in range(2):
                    nc.tensor.matmul(ps, yb[:, kc, k * SUB:(k + 1) * SUB],
                                     wprojT[kc], start=(kc == 0), stop=(kc == 1))
                x1t = x1p.tile([128, C], dt.bfloat16, tag="x1", name="x1")
                nc.vector.tensor_add(out=x1t, in0=x_tiles[k], in1=ps)
                x1_tiles.append(x1t)
                ln_stats(x1t, mv2, k)
            state[("x1", slot, w)] = x1_tiles
            state[("mv2", slot)] = mv2

        def dn_group(slot, r, w, k, fftiles, x1_tiles):
            t0 = w * WIN + k * SUB
            pd = acc_ps.tile([128, C], dt.float32, tag="acc", name="dn")
            for fc in range(8):
                nc.tensor.matmul(pd, fftiles[fc][:, k * SUB:(k + 1) * SUB],
                                 w2T[fc], start=(fc == 0), stop=(fc == 7))
            ot = outp.tile([128, C], dt.float32, tag="o", name="o")
            nc.vector.tensor_add(out=ot, in0=x1_tiles[k], in1=pd)
            nc.sync.dma_start(out=out_d[r, t0:t0 + SUB, :], in_=ot)

        def ln2_phase(slot, w):
            x1_tiles = state[("x1", slot, w)]
            mv2 = state[("mv2", slot)]
            rs2 = ln_batch_rsqrt(mv2)
            h2 = h2b.tile([128, 2, WIN], up_dt, tag=f"h2dr{slot}",
                          name=f"h2dr{slot}")
            for kp in range(2):
                ln_pair(x1_tiles, mv2, rs2, kp, h2, kp * 2 * SUB, WIN,
                        EG["h2_copy"])
            state[("h2", slot)] = h2

        def up_pairs(slot, r, w, fillers):
            """FFN-up for w; `fillers` (LN1 pieces of w+1, x loads of w+2)
            are emitted between up pairs to keep the PE stream dense."""
            h2 = state[("h2", slot)]
            fftiles = []
            fill = iter(fillers)
            for p in range(4):  # fc pairs
                pu = up_ps.tile([128, 8 * SUB], dt.float32, tag="up", name="up")
                for half in range(2):
                    fc = 2 * p + half
                    if FP8_UP:
                        nc.tensor.matmul(pu[:, half * WIN:(half + 1) * WIN],
                                         w1dr[:, :, 128 * fc:128 * (fc + 1)], h2,
                                         start=True, stop=True, perf_mode=DR)
                    else:
                        nc.tensor.matmul(pu[:, half * WIN:(half + 1) * WIN],
                                         w1dr[:, 0, 128 * fc:128 * (fc + 1)],
                                         h2[:, 0, :], start=True, stop=False)
                        nc.tensor.matmul(pu[:, half * WIN:(half + 1) * WIN],
                                         w1dr[:, 1, 128 * fc:128 * (fc + 1)],
                                         h2[:, 1, :], start=False, stop=True)
                fb = ffb.tile([128, 2, WIN], dt.bfloat16, tag=f"ffp{slot}{p}",
                              name=f"ffp{slot}{p}")
                # b1_eff == 0 (asserted in _prep): relu(pu/16)
                nc.scalar.activation(fb, pu, AF.Relu, bias=0.0,
                                     scale=1.0 / UP_W_SCALE)
                fftiles.append(fb)
                f = next(fill, None)
                if f is not None:
                    f()
            for f in fill:
                f()
            ffv = []
            for p in range(4):
                for half in range(2):
                    ffv.append(fftiles[p][:, half, :])
            state[("ff", slot)] = (r, w, ffv, state[("x1", slot, w)])

        def dn_piece(slot, k):
            pv = state[("ff", slot)]
            if pv is not None:
                dn_group(slot, pv[0], pv[1], k, pv[2], pv[3])

        # ---- main pipeline ----
        # Fine-grained per-window emission: the two slots' phases are woven
        # so PE-dense blocks (stage1, dn, up pairs) cover the other slot's
        # LN chains and psum->sbuf copy latencies.
        nslots = min(2, n_rows)
        for rp in range(0, n_rows, nslots):
            hBs = []
            for slot in range(nslots):
                hB = hb_pool.tile([128, 2, PADW + t_len], att_dt,
                                  tag=f"hb{slot}", name=f"hb{slot}")
                nc.gpsimd.memset(hB[:, :, 0:PADW], 0.0)
                hBs.append(hB)
            for slot in range(nslots):
                state[("ff", slot)] = None
                r = rp + slot
                if not state.get(("pref", rp)):
                    x_load(slot, r, 0)
                    x_load(slot, r, 1)
                ln1_piece(slot, hBs[slot], 0, 0)
                ln1_piece(slot, hBs[slot], 0, 1)
            s0, s1 = 0, 1 % nslots
            for w in range(nwin):
                r0, r1 = rp + s0, rp + s1
                stage1_phase(s0, hBs[s0], r0, w)
                dn_piece(s0, 0)
                dn_piece(s0, 1)
                stage2_phase(s0, r0, w)
                dn_piece(s0, 2)
                if nslots > 1:
                    stage1_phase(s1, hBs[s1], r1, w)
                ln2_phase(s0, w)
                if nslots > 1:
                    stage2_phase(s1, r1, w)
                fillers = [lambda slot=s0: dn_piece(slot, 3)]
                if w + 1 < nwin:
                    for kp in range(2):
                        fillers.append(lambda slot=s0, w=w, kp=kp:
                                       ln1_piece(slot, hBs[slot], w + 1, kp))
                if w + 2 < nwin:
                    fillers.append(lambda slot=s0, r=r0, w=w:
                                   x_load(slot, r, w + 2))
                up_pairs(s0, r0, w, fillers)
                if nslots > 1:
                    dn_piece(s1, 0)
                    dn_piece(s1, 1)
                    ln2_phase(s1, w)
                    dn_piece(s1, 2)
                    fillers = [lambda slot=s1: dn_piece(slot, 3)]
                    if w + 1 < nwin:
                        for kp in range(2):
                            fillers.append(lambda slot=s1, w=w, kp=kp:
                                           ln1_piece(slot, hBs[slot], w + 1, kp))
                    if w + 2 < nwin:
                        fillers.append(lambda slot=s1, r=r1, w=w:
                                       x_load(slot, r, w + 2))
                    up_pairs(s1, r1, w, fillers)
                if w == nwin - 1 and rp + nslots < n_rows:
                    # prefetch the next row-pair's first x windows (independent
                    # DMA + stats) so the rp-boundary prologue is compute-only
                    for slot in range(nslots):
                        x_load(slot, rp + nslots + slot, 0)
                        x_load(slot, rp + nslots + slot, 1)
                    state[("pref", rp + nslots)] = True
            for slot in range(nslots):
                r_, w_, ff_, x1_ = state[("ff", slot)]
                for k in range(4):
                    dn_group(slot, r_, w_, k, ff_, x1_)
                state[("ff", slot)] = None

    nc.compile()
    return nc


_CACHE = {}


def _get_nc():
    if "nc" not in _CACHE:
        _CACHE["nc"] = _build()
    return _CACHE["nc"]


def _run(inputs, trace_dir=None):
    from concourse.bass_utils import run_bass_kernel_spmd
    from concourse import bass2jax

    x = np.asarray(inputs["x"], dtype=np.float32)
    w = _prep(inputs)
    nc = _get_nc()

    in_maps = []
    for core in range(NCORES):
        m = dict(w)
        m["x"] = np.ascontiguousarray(x[core * BPC:(core + 1) * BPC])
        in_maps.append(m)

    if trace_dir is None:
        res = run_bass_kernel_spmd(nc, in_maps, list(range(NCORES)))
        results, exec_ns = res.results, None
    else:
        import ctypes
        from contextlib import contextmanager

        lib = ctypes.CDLL("/opt/axon/libaxon_pjrt.so")
        lib.axon_start_nrt_profile.argtypes = [
            ctypes.POINTER(ctypes.c_int64), ctypes.c_size_t]
        lib.axon_start_nrt_profile.restype = ctypes.c_int64
        lib.axon_stop_nrt_profile.argtypes = [ctypes.c_char_p]
        lib.axon_stop_nrt_profile.restype = ctypes.c_int64

        @contextmanager
        def hook(output_dir, device_ids):
            import jax
            jax.devices()
            ids = (ctypes.c_int64 * len(device_ids))(*device_ids)
            rc = lib.axon_start_nrt_profile(ids, len(device_ids))
            if rc != 0:
                raise RuntimeError(f"axon_start_nrt_profile rc={rc}")
            try:
                yield
            finally:
                n = lib.axon_stop_nrt_profile(str(output_dir).encode())
                print(f"profile: {n} file(s) written to {output_dir}")

        os.makedirs(trace_dir, exist_ok=True)
        with hook(trace_dir, [0]):
            results = bass2jax.run_bass_via_pjrt(nc, in_maps, n_cores=NCORES)
        exec_ns = None  # caller post-processes the NTFFs

    out = np.concatenate([np.asarray(results[i]["out"]) for i in range(NCORES)], axis=0)
    return out, exec_ns


def kernel(**inputs):
    out, _ = _run(inputs)
    return out


# revision 32
# speedup vs baseline: 1.0055x; 1.0055x over previous
"""Trainium2 Bass kernel for nn_Block_627065225827 (dense_transformer).

Self-contained: hardcodes shapes B=32, T=4096, C=256, H=8 and the
data-parallel-over-batch sharding (4 batch rows per core, 8 cores).

Math (see reference):
    h   = LN1(x) * g1 + b1ln
    id  = h @ w_id.T ;  inf = h @ w_inf.T            (per-head view [H, hs])
    inf = inf / (1+K);  shifted[t] = inf[t - s_h]    (zero for t < s_h)
    sa  = (K/(1+K) * id + shifted) @ w_proj.T + b_proj
    x1  = x + sa
    ff  = relu(LN2(x1)*g2+b2ln @ w1.T + b1) @ w2.T + b2
    out = x1 + ff

Two-stage attention (rank-structure, exact):
    y[c,t] = (a . w_id_g h)[c,t] + (b . w_inf_g h)[c, t - s_c]
    sa     = w_proj_perm @ y
  with all 256 y-channels permuted so shift groups are contiguous
  (band of 64 per shift s=1..4). The per-band temporal shift is applied
  by accumulating each band's matmul into PSUM with a shifted rhs token
  window read from hB (channel-major h-hat, 16 zero pad columns at
  head). id (and the base-partition-0 inf bands) run fp8 DoubleRow;
  offset-64 bands run normal-mode fp8 (DR requires dst partition 0).
  FFN up runs fp8 DoubleRow with paired psum tiles ([128,1024] across
  two banks) and one relu per pair. Weights are pre-scaled (x64 attn,
  x16 up), compensated in the psum->sbuf copy / relu input scale.
  x and x1 are kept in bf16 (cast on DMA load); the residual-path
  quantization (~0.4% of |out|) fits the error budget.

Schedule: 2 batch rows interleaved; x loads + LN1 stats prefetched two
windows ahead; LN1-apply of w+1 and the down-proj of w-1 are emitted
as fillers between the up-proj pairs of w to keep the PE stream dense
(HAM warm) and spread V/S queue pressure.
"""

import os
from contextlib import ExitStack

import numpy as np
import ml_dtypes

B, T, C, H = 32, 4096, 256, 8
HS = C // H
NCORES = 8
BPC = B // NCORES  # batch rows per core
SHIFTS = [1, 2, 3, 4, 1, 2, 3, 4]
EPS = 1e-5
PADW = 16  # zero columns at the head of hB (keeps DoubleRow plane step %16)
WIN = 512  # tokens per window
SUB = 128  # tokens per subtile (partition dim)

_f64 = np.float64
_bf16 = ml_dtypes.bfloat16
_fp8 = ml_dtypes.float8_e4m3

FP8_ATT = True   # stage1 (id/inf) in fp8 (id + base-0 bands DoubleRow)
FP8_UP = True    # FFN up in fp8 DoubleRow
ATT_W_SCALE = 64.0
UP_W_SCALE = 16.0
RELU_PAIR = True  # one relu per up-pair ([128,1024] across 2 psum banks)

# engine for flexible psum->sbuf copies: 'v' | 's'
EG = {
    "hb_copy": "s",  # merged transpose psum -> hB (fp8 cast), 2/window
    "h2_copy": "s",  # merged transpose psum -> h2 (fp8 cast), 2/window
    "y_copy": "s",   # stage1 psum -> yB bf16 (scale 1/ATT_W_SCALE)
}
HN_S = 0  # how many of the 4 per-window-slot hn tensor_scalars run on scalar

# permutation of the 256 y channels: group by shift (s=1..4, 64 ch each)
PERM = np.concatenate(
    [np.concatenate([np.arange(h * HS, (h + 1) * HS) for h in range(H)
                     if SHIFTS[h] == s]) for s in (1, 2, 3, 4)]
)


def _prep(inputs):
    """Fold LN gains/biases + per-head scalars into the weights (host, numpy)."""
    g = {k: np.asarray(v, dtype=_f64) for k, v in inputs.items() if k != "x"}
    K = np.exp(g["khead"])  # [H]
    a_row = np.repeat(K / (1.0 + K), HS)  # [C] per id-output channel
    b_row = np.repeat(1.0 / (1.0 + K), HS)  # [C] per inf-output channel

    w_id_g = g["w_id"] * g["ln1_g"][None, :]
    w_inf_g = g["w_inf"] * g["ln1_g"][None, :]
    w_id_s = (w_id_g * a_row[:, None])[PERM, :]   # permuted rows
    w_inf_s = (w_inf_g * b_row[:, None])[PERM, :]
    w_proj_p = g["w_proj"][:, PERM]

    # LN bias / proj / ffn bias constants must be zero for this kernel
    # structure (true for this problem instance).
    assert np.abs(g["ln1_b"]).max() == 0 and np.abs(g["b_proj"]).max() == 0
    assert np.abs(g["ln2_b"]).max() == 0 and np.abs(g["b1"]).max() == 0
    assert np.abs(g["b2"]).max() == 0

    att_dt = _fp8 if FP8_ATT else _bf16
    att_s = ATT_W_SCALE if FP8_ATT else 1.0

    def dr_pack(wT):  # [C_in, M] -> [128, 2, M] (contraction = 128*j + k)
        Min = wT.shape[1]
        return np.ascontiguousarray(wT.reshape(2, 128, Min).transpose(1, 0, 2))

    w1_g = g["w1"] * g["ln2_g"][None, :]
    w1dr = dr_pack(w1_g.T * UP_W_SCALE)  # [128, 2, 4C]

    out = {
        "widT": dr_pack(w_id_s.T * att_s).astype(att_dt),
        "winfT": dr_pack(w_inf_s.T * att_s).astype(att_dt),
        "wprojT": np.ascontiguousarray(w_proj_p.T.reshape(2, 128, C)).astype(_bf16),
        "w1dr": np.ascontiguousarray(w1dr).astype(_fp8 if FP8_UP else _bf16),
        "w2T": np.ascontiguousarray(g["w2"].T.reshape(8, 128, C)).astype(_bf16),
    }
    return out


def _build(n_rows=BPC, t_len=T):
    """Build + compile the per-core Bass program. Returns the finalized nc."""
    import concourse.bacc as bacc
    import concourse.bass as bass
    import concourse.mybir as mybir
    import concourse.tile as tile
    from concourse.masks import make_identity

    dt = mybir.dt
    AF = mybir.ActivationFunctionType
    OP = mybir.AluOpType
    DR = mybir.MatmulPerfMode.DoubleRow

    att_dt = dt.float8e4 if FP8_ATT else dt.bfloat16
    up_dt = dt.float8e4 if FP8_UP else dt.bfloat16

    nwin = t_len // WIN
    nc = bacc.Bacc("TRN2", target_bir_lowering=False, debug=False, num_devices=NCORES)

    x_d = nc.declare_dram_parameter("x", [n_rows, t_len, C], dt.float32, isOutput=False)
    widT_d = nc.declare_dram_parameter("widT", [128, 2, C], att_dt, isOutput=False)
    winfT_d = nc.declare_dram_parameter("winfT", [128, 2, C], att_dt, isOutput=False)
    wprojT_d = nc.declare_dram_parameter("wprojT", [2, 128, C], dt.bfloat16, isOutput=False)
    w1dr_d = nc.declare_dram_parameter("w1dr", [128, 2, 4 * C], up_dt, isOutput=False)
    w2T_d = nc.declare_dram_parameter("w2T", [8, 128, C], dt.bfloat16, isOutput=False)
    out_d = nc.declare_dram_parameter("out", [n_rows, t_len, C], dt.float32, isOutput=True)

    with tile.TileContext(nc) as tc, ExitStack() as ctx:
        singles = ctx.enter_context(tc.tile_pool(name="singles", bufs=1))
        hb_pool = ctx.enter_context(tc.tile_pool(name="hb", bufs=1))
        xin = ctx.enter_context(tc.tile_pool(name="xin", bufs=8))
        x1p = ctx.enter_context(tc.tile_pool(name="x1p", bufs=16))
        hnorm = ctx.enter_context(tc.tile_pool(name="hnorm", bufs=12))
        stats = ctx.enter_context(tc.tile_pool(name="stats", bufs=24))
        ybp = ctx.enter_context(tc.tile_pool(name="ybp", bufs=3))
        h2b = ctx.enter_context(tc.tile_pool(name="h2b", bufs=3))
        ffb = ctx.enter_context(tc.tile_pool(name="ffb", bufs=2))
        outp = ctx.enter_context(tc.tile_pool(name="outp", bufs=10))
        tp_ps = ctx.enter_context(tc.tile_pool(name="tp_ps", bufs=1, space="PSUM"))
        acc_ps = ctx.enter_context(tc.tile_pool(name="acc_ps", bufs=2, space="PSUM"))
        y_ps = ctx.enter_context(tc.tile_pool(name="y_ps", bufs=1, space="PSUM"))
        up_ps = ctx.enter_context(tc.tile_pool(name="up_ps", bufs=2, space="PSUM"))

        # ---- constants / weights in SBUF ----
        ident = singles.tile([128, 128], dt.bfloat16)
        make_identity(nc, ident)
        eps_t = singles.tile([128, 1], dt.float32)
        nc.vector.memset(eps_t, EPS)
        widT = singles.tile([128, 2, C], att_dt, tag="widT")
        nc.sync.dma_start(out=widT, in_=widT_d[:, :, :])
        winfT = singles.tile([128, 2, C], att_dt, tag="winfT")
        nc.sync.dma_start(out=winfT, in_=winfT_d[:, :, :])
        wprojT = []
        for kc in range(2):
            w = singles.tile([128, C], dt.bfloat16, tag=f"wprojT{kc}")
            nc.sync.dma_start(out=w, in_=wprojT_d[kc])
            wprojT.append(w)
        w1dr = singles.tile([128, 2, 4 * C], up_dt, tag="w1dr")
        nc.sync.dma_start(out=w1dr, in_=w1dr_d[:, :, :])
        w2T = []
        for fc in range(8):
            w = singles.tile([128, C], dt.bfloat16, tag=f"w2T{fc}")
            nc.sync.dma_start(out=w, in_=w2T_d[fc])
            w2T.append(w)

        def ln_stats(src_tile, mvW, k):
            st = stats.tile([128, 6], dt.float32, tag="st", name="st")
            nc.vector.bn_stats(st, src_tile)
            nc.vector.bn_aggr(mvW[:, 2 * k:2 * k + 2], st)

        def ln_batch_rsqrt(mvW):
            sdW = stats.tile([128, 4], dt.float32, tag="sd", name="sd")
            var_view = bass.AP(tensor=mvW.tensor, offset=mvW.offset + 1,
                               ap=[mvW.ap[0], [2, 4]])
            nc.scalar.activation(sdW, var_view, AF.Sqrt, bias=eps_t, scale=1.0)
            rsW = stats.tile([128, 4], dt.float32, tag="rs", name="rs")
            nc.vector.reciprocal(rsW, sdW)
            return rsW

        def ln_pair(src_tiles, mvW, rsW, kp, dst_tile, dst_col, dst_cstride,
                    copy_eng):
            """LN-apply + transpose two token subtiles (k = 2*kp, 2*kp+1) and
            copy the merged [128, 512] psum into the channel-major dst."""
            pt = tp_ps.tile([128, 4 * SUB], dt.float32, tag="tp", name="tp")
            for kk in range(2):
                k = 2 * kp + kk
                hn = hnorm.tile([128, C], dt.bfloat16, tag="hn", name="hn")
                nc.vector.tensor_scalar(
                    out=hn, in0=src_tiles[k], scalar1=mvW[:, 2 * k:2 * k + 1],
                    scalar2=rsW[:, k:k + 1], op0=OP.subtract, op1=OP.mult,
                )
                for c in range(2):
                    nc.tensor.matmul(
                        pt[:, (2 * kk + c) * SUB:(2 * kk + c + 1) * SUB],
                        hn[:, 128 * c:128 * (c + 1)], ident,
                        start=True, stop=True)
            src4 = bass.AP(tensor=pt.tensor, offset=pt.offset,
                           ap=[pt.ap[0], [SUB, 2], [2 * SUB, 2], [1, SUB]])
            dst4 = bass.AP(tensor=dst_tile.tensor,
                           offset=dst_tile.offset + dst_col,
                           ap=[dst_tile.ap[0], [dst_cstride, 2], [SUB, 2], [1, SUB]])
            if copy_eng == "s":
                nc.scalar.copy(out=dst4, in_=src4)
            else:
                nc.vector.tensor_copy(out=dst4, in_=src4)

        state = {}

        def x_load(slot, r, w):
            """One DMA (f32->bf16 cast) for window w's 4 x subtiles + LN1 stats.
            SBUF layout [128, 4, C]: token t0w + 128*k + p at [p, k, :]."""
            t0w = w * WIN
            xw = xin.tile([128, 4, C], dt.bfloat16, tag="x", name="x")
            src = x_d[r, t0w:t0w + WIN, :].rearrange("(k p) c -> p k c", p=128)
            nc.gpsimd.dma_start(out=xw, in_=src)
            x_tiles = [xw[:, k, :] for k in range(4)]
            mv1 = stats.tile([128, 8], dt.float32, tag="mv1", name="mv1")
            for k in range(4):
                ln_stats(x_tiles[k], mv1, k)
            state[("x", slot, w)] = x_tiles
            state[("mv1", slot, w)] = mv1

        def ln1_piece(slot, hB, w, kp):
            """LN1-apply + transpose of k-pair kp of window w into hB."""
            if kp == 0:
                state[("rs1", slot, w)] = ln_batch_rsqrt(state[("mv1", slot, w)])
            ln_pair(state[("x", slot, w)], state[("mv1", slot, w)],
                    state[("rs1", slot, w)], kp, hB,
                    PADW + w * WIN + kp * 2 * SUB, PADW + t_len, EG["hb_copy"])

        def stage1_phase(slot, hB, r, w):
            col0 = PADW + w * WIN
            yb = ybp.tile([128, 2, WIN], dt.bfloat16, tag=f"yb{slot}",
                          name=f"yb{slot}")
            state[("yb", slot)] = yb
            for mc in range(2):
                yp = y_ps.tile([128, WIN], dt.float32, tag="y", name="y")
                if FP8_ATT:
                    # id: full 128-partition dst -> DoubleRow ok.
                    nc.tensor.matmul(yp, widT[:, :, 128 * mc:128 * (mc + 1)],
                                     hB[:, :, col0:col0 + WIN],
                                     start=True, stop=False, perf_mode=DR)
                    # band 0 dst is partition base 0 -> DoubleRow ok
                    s = 2 * mc + 1
                    nc.tensor.matmul(
                        yp[0:64, :], winfT[:, :, 128 * mc:128 * mc + 64],
                        hB[:, :, col0 - s:col0 - s + WIN],
                        start=False, stop=False, perf_mode=DR)
                    # band 1 dst at partition 64: DR invalid -> normal fp8
                    s = 2 * mc + 2
                    for kc in range(2):
                        nc.tensor.matmul(
                            yp[64:128, :],
                            winfT[:, kc, 128 * mc + 64:128 * mc + 128],
                            hB[:, kc, col0 - s:col0 - s + WIN],
                            start=False, stop=(kc == 1))
                else:
                    for kc in range(2):
                        nc.tensor.matmul(yp, widT[:, kc, 128 * mc:128 * (mc + 1)],
                                         hB[:, kc, col0:col0 + WIN],
                                         start=(kc == 0), stop=False)
                    for b in range(2):
                        s = 2 * mc + b + 1
                        for kc in range(2):
                            nc.tensor.matmul(
                                yp[64 * b:64 * (b + 1), :],
                                winfT[:, kc, 128 * mc + 64 * b:128 * mc + 64 * (b + 1)],
                                hB[:, kc, col0 - s:col0 - s + WIN],
                                start=False, stop=(b == 1 and kc == 1))
                yscale = 1.0 / ATT_W_SCALE if FP8_ATT else 1.0
                nc.scalar.mul(yb[:, mc, :], yp, yscale)

        def stage2_phase(slot, r, w):
            x_tiles = state[("x", slot, w)]
            yb = state[("yb", slot)]
            x1_tiles = []
            mv2 = stats.tile([128, 8], dt.float32, tag="mv2", name="mv2")
            for k in range(4):
                ps = acc_ps.tile([128, C], dt.float32, tag="acc", name="sa")
                for kc in range(2):
                    nc.tensor.matmul(ps, yb[:, kc, k * SUB:(k + 1) * SUB],
                                     wprojT[kc], start=(kc == 0), stop=(kc == 1))
                x1t = x1p.tile([128, C], dt.bfloat16, tag="x1", name="x1")
                nc.vector.tensor_add(out=x1t, in0=x_tiles[k], in1=ps)
                x1_tiles.append(x1t)
                ln_stats(x1t, mv2, k)
            state[("x1", slot, w)] = x1_tiles
            state[("mv2", slot)] = mv2

        def dn_group(slot, r, w, k, fftiles, x1_tiles):
            t0 = w * WIN + k * SUB
            pd = acc_ps.tile([128, C], dt.float32, tag="acc", name="dn")
            for fc in range(8):
                nc.tensor.matmul(pd, fftiles[fc][:, k * SUB:(k + 1) * SUB],
                                 w2T[fc], start=(fc == 0), stop=(fc == 7))
            ot = outp.tile([128, C], dt.float32, tag="o", name="o")
            nc.vector.tensor_add(out=ot, in0=x1_tiles[k], in1=pd)
            nc.sync.dma_start(out=out_d[r, t0:t0 + SUB, :], in_=ot)

        def ln2_phase(slot, w):
            x1_tiles = state[("x1", slot, w)]
            mv2 = state[("mv2", slot)]
            rs2 = ln_batch_rsqrt(mv2)
            h2 = h2b.tile([128, 2, WIN], up_dt, tag=f"h2dr{slot}",
                          name=f"h2dr{slot}")
            for kp in range(2):
                ln_pair(x1_tiles, mv2, rs2, kp, h2, kp * 2 * SUB, WIN,
                        EG["h2_copy"])
            state[("h2", slot)] = h2

        def up_pairs(slot, r, w, fillers):
            """FFN-up for w; `fillers` (LN1 pieces of w+1, x loads of w+2)
            are emitted between up pairs to keep the PE stream dense."""
            h2 = state[("h2", slot)]
            fftiles = []
            fill = iter(fillers)
            for p in range(4):  # fc pairs
                pu = up_ps.tile([128, 8 * SUB], dt.float32, tag="up", name="up")
                for half in range(2):
                    fc = 2 * p + half
                    if FP8_UP:
                        nc.tensor.matmul(pu[:, half * WIN:(half + 1) * WIN],
                                         w1dr[:, :, 128 * fc:128 * (fc + 1)], h2,
                                         start=True, stop=True, perf_mode=DR)
                    else:
                        nc.tensor.matmul(pu[:, half * WIN:(half + 1) * WIN],
                                         w1dr[:, 0, 128 * fc:128 * (fc + 1)],
                                         h2[:, 0, :], start=True, stop=False)
                        nc.tensor.matmul(pu[:, half * WIN:(half + 1) * WIN],
                                         w1dr[:, 1, 128 * fc:128 * (fc + 1)],
                                         h2[:, 1, :], start=False, stop=True)
                fb = ffb.tile([128, 2, WIN], dt.bfloat16, tag=f"ffp{slot}{p}",
                              name=f"ffp{slot}{p}")
                # b1_eff == 0 (asserted in _prep): relu(pu/16)
                nc.scalar.activation(fb, pu, AF.Relu, bias=0.0,
                                     scale=1.0 / UP_W_SCALE)
                fftiles.append(fb)
                f = next(fill, None)
                if f is not None:
                    f()
            for f in fill:
                f()
            ffv = []
            for p in range(4):
                for half in range(2):
                    ffv.append(fftiles[p][:, half, :])
            state[("ff", slot)] = (r, w, ffv, state[("x1", slot, w)])

        def dn_piece(slot, k):
            pv = state[("ff", slot)]
            if pv is not None:
                dn_group(slot, pv[0], pv[1], k, pv[2], pv[3])

        # ---- main pipeline ----
        # Fine-grained per-window emission: the two slots' phases are woven
        # so PE-dense blocks (stage1, dn, up pairs) cover the other slot's
        # LN chains and psum->sbuf copy latencies.
        nslots = min(2, n_rows)
        for rp in range(0, n_rows, nslots):
            hBs = []
            for slot in range(nslots):
                hB = hb_pool.tile([128, 2, PADW + t_len], att_dt,
                                  tag=f"hb{slot}", name=f"hb{slot}")
                nc.gpsimd.memset(hB[:, :, 0:PADW], 0.0)
                hBs.append(hB)
            for slot in range(nslots):
                state[("ff", slot)] = None
                r = rp + slot
                x_load(slot, r, 0)
                x_load(slot, r, 1)
                ln1_piece(slot, hBs[slot], 0, 0)
                ln1_piece(slot, hBs[slot], 0, 1)
            s0, s1 = 0, 1 % nslots
            for w in range(nwin):
                r0, r1 = rp + s0, rp + s1
                stage1_phase(s0, hBs[s0], r0, w)
                dn_piece(s0, 0)
                dn_piece(s0, 1)
                stage2_phase(s0, r0, w)
                dn_piece(s0, 2)
                if nslots > 1:
                    stage1_phase(s1, hBs[s1], r1, w)
                ln2_phase(s0, w)
                if nslots > 1:
                    stage2_phase(s1, r1, w)
                fillers = [lambda slot=s0: dn_piece(slot, 3)]
                if w + 1 < nwin:
                    for kp in range(2):
                        fillers.append(lambda slot=s0, w=w, kp=kp:
                                       ln1_piece(slot, hBs[slot], w + 1, kp))
                if w + 2 < nwin:
                    fillers.append(lambda slot=s0, r=r0, w=w:
                                   x_load(slot, r, w + 2))
                up_pairs(s0, r0, w, fillers)
                if nslots > 1:
                    dn_piece(s1, 0)
                    dn_piece(s1, 1)
                    ln2_phase(s1, w)
                    dn_piece(s1, 2)
                    fillers = [lambda slot=s1: dn_piece(slot, 3)]
                    if w + 1 < nwin:
                        for kp in range(2):
                            fillers.append(lambda slot=s1, w=w, kp=kp:
                                           ln1_piece(slot, hBs[slot], w + 1, kp))
                    if w + 2 < nwin:
                        fillers.append(lambda slot=s1, r=r1, w=w:
                                       x_load(slot, r, w + 2))
                    up_pairs(s1, r1, w, fillers)
            for slot in range(nslots):
                r_, w_, ff_, x1_ = state[("ff", slot)]
                for k in range(4):
                    dn_group(slot, r_, w_, k, ff_, x1_)
                state[("ff", slot)] = None

    nc.compile()
    return nc


_CACHE = {}


def _get_nc():
    if "nc" not in _CACHE:
        _CACHE["nc"] = _build()
    return _CACHE["nc"]


def _run(inputs, trace_dir=None):
    from concourse.bass_utils import run_bass_kernel_spmd
    from concourse import bass2jax

    x = np.asarray(inputs["x"], dtype=np.float32)
    w = _prep(inputs)
    nc = _get_nc()

    in_maps = []
    for core in range(NCORES):
        m = dict(w)
        m["x"] = np.ascontiguousarray(x[core * BPC:(core + 1) * BPC])
        in_maps.append(m)

    if trace_dir is None:
        res = run_bass_kernel_spmd(nc, in_maps, list(range(NCORES)))
        results, exec_ns = res.results, None
    else:
        import ctypes
        from contextlib import contextmanager

        lib = ctypes.CDLL("/opt/axon/libaxon_pjrt.so")
        lib.axon_start_nrt_profile.argtypes = [
            ctypes.POINTER(ctypes.c_int64), ctypes.c_size_t]
        lib.axon_start_nrt_profile.restype = ctypes.c_int64
        lib.axon_stop_nrt_profile.argtypes = [ctypes.c_char_p]
        lib.axon_stop_nrt_profile.restype = ctypes.c_int64

        @contextmanager
        def hook(output_dir, device_ids):
            import jax
            jax.devices()
            ids = (ctypes.c_int64 * len(device_ids))(*device_ids)
            rc = lib.axon_start_nrt_profile(ids, len(device_ids))
            if rc != 0:
                raise RuntimeError(f"axon_start_nrt_profile rc={rc}")
            try:
                yield
            finally:
                n = lib.axon_stop_nrt_profile(str(output_dir).encode())
                print(f"profile: {n} file(s) written to {output_dir}")

        os.makedirs(trace_dir, exist_ok=True)
        with hook(trace_dir, [0]):
            results = bass2jax.run_bass_via_pjrt(nc, in_maps, n_cores=NCORES)
        exec_ns = None  # caller post-processes the NTFFs

    out = np.concatenate([np.asarray(results[i]["out"]) for i in range(NCORES)], axis=0)
    return out, exec_ns


def kernel(**inputs):
    out, _ = _run(inputs)
    return out


# revision 33
# speedup vs baseline: 1.0111x; 1.0056x over previous
"""Trainium2 Bass kernel for nn_Block_627065225827 (dense_transformer).

Self-contained: hardcodes shapes B=32, T=4096, C=256, H=8 and the
data-parallel-over-batch sharding (4 batch rows per core, 8 cores).

Math (see reference):
    h   = LN1(x) * g1 + b1ln
    id  = h @ w_id.T ;  inf = h @ w_inf.T            (per-head view [H, hs])
    inf = inf / (1+K);  shifted[t] = inf[t - s_h]    (zero for t < s_h)
    sa  = (K/(1+K) * id + shifted) @ w_proj.T + b_proj
    x1  = x + sa
    ff  = relu(LN2(x1)*g2+b2ln @ w1.T + b1) @ w2.T + b2
    out = x1 + ff

Two-stage attention (rank-structure, exact):
    y[c,t] = (a . w_id_g h)[c,t] + (b . w_inf_g h)[c, t - s_c]
    sa     = w_proj_perm @ y
  with all 256 y-channels permuted so shift groups are contiguous
  (band of 64 per shift s=1..4). The per-band temporal shift is applied
  by accumulating each band's matmul into PSUM with a shifted rhs token
  window read from hB (channel-major h-hat, 16 zero pad columns at
  head). id (and the base-partition-0 inf bands) run fp8 DoubleRow;
  offset-64 bands run normal-mode fp8 (DR requires dst partition 0).
  FFN up runs fp8 DoubleRow with paired psum tiles ([128,1024] across
  two banks) and one relu per pair. Weights are pre-scaled (x64 attn,
  x16 up), compensated in the psum->sbuf copy / relu input scale.
  x and x1 are kept in bf16 (cast on DMA load); the residual-path
  quantization (~0.4% of |out|) fits the error budget.

Schedule: 2 batch rows interleaved; x loads + LN1 stats prefetched two
windows ahead; LN1-apply of w+1 and the down-proj of w-1 are emitted
as fillers between the up-proj pairs of w to keep the PE stream dense
(HAM warm) and spread V/S queue pressure.
"""

import os
from contextlib import ExitStack

import numpy as np
import ml_dtypes

B, T, C, H = 32, 4096, 256, 8
HS = C // H
NCORES = 8
BPC = B // NCORES  # batch rows per core
SHIFTS = [1, 2, 3, 4, 1, 2, 3, 4]
EPS = 1e-5
PADW = 16  # zero columns at the head of hB (keeps DoubleRow plane step %16)
WIN = 512  # tokens per window
SUB = 128  # tokens per subtile (partition dim)

_f64 = np.float64
_bf16 = ml_dtypes.bfloat16
_fp8 = ml_dtypes.float8_e4m3

FP8_ATT = True   # stage1 (id/inf) in fp8 (id + base-0 bands DoubleRow)
FP8_UP = True    # FFN up in fp8 DoubleRow
ATT_W_SCALE = 64.0
UP_W_SCALE = 16.0
RELU_PAIR = True  # one relu per up-pair ([128,1024] across 2 psum banks)

# engine for flexible psum->sbuf copies: 'v' | 's'
EG = {
    "hb_copy": "s",  # merged transpose psum -> hB (fp8 cast), 2/window
    "h2_copy": "s",  # merged transpose psum -> h2 (fp8 cast), 2/window
    "y_copy": "s",   # stage1 psum -> yB bf16 (scale 1/ATT_W_SCALE)
}
HN_S = 0  # how many of the 4 per-window-slot hn tensor_scalars run on scalar

# permutation of the 256 y channels: group by shift (s=1..4, 64 ch each)
PERM = np.concatenate(
    [np.concatenate([np.arange(h * HS, (h + 1) * HS) for h in range(H)
                     if SHIFTS[h] == s]) for s in (1, 2, 3, 4)]
)


def _prep(inputs):
    """Fold LN gains/biases + per-head scalars into the weights (host, numpy)."""
    g = {k: np.asarray(v, dtype=_f64) for k, v in inputs.items() if k != "x"}
    K = np.exp(g["khead"])  # [H]
    a_row = np.repeat(K / (1.0 + K), HS)  # [C] per id-output channel
    b_row = np.repeat(1.0 / (1.0 + K), HS)  # [C] per inf-output channel

    w_id_g = g["w_id"] * g["ln1_g"][None, :]
    w_inf_g = g["w_inf"] * g["ln1_g"][None, :]
    w_id_s = (w_id_g * a_row[:, None])[PERM, :]   # permuted rows
    w_inf_s = (w_inf_g * b_row[:, None])[PERM, :]
    w_proj_p = g["w_proj"][:, PERM]

    # LN bias / proj / ffn bias constants must be zero for this kernel
    # structure (true for this problem instance).
    assert np.abs(g["ln1_b"]).max() == 0 and np.abs(g["b_proj"]).max() == 0
    assert np.abs(g["ln2_b"]).max() == 0 and np.abs(g["b1"]).max() == 0
    assert np.abs(g["b2"]).max() == 0

    att_dt = _fp8 if FP8_ATT else _bf16
    att_s = ATT_W_SCALE if FP8_ATT else 1.0

    def dr_pack(wT):  # [C_in, M] -> [128, 2, M] (contraction = 128*j + k)
        Min = wT.shape[1]
        return np.ascontiguousarray(wT.reshape(2, 128, Min).transpose(1, 0, 2))

    w1_g = g["w1"] * g["ln2_g"][None, :]
    w1dr = dr_pack(w1_g.T * UP_W_SCALE)  # [128, 2, 4C]

    out = {
        "widT": dr_pack(w_id_s.T * att_s).astype(att_dt),
        "winfT": dr_pack(w_inf_s.T * att_s).astype(att_dt),
        "wprojT": np.ascontiguousarray(w_proj_p.T.reshape(2, 128, C)).astype(_bf16),
        "w1dr": np.ascontiguousarray(w1dr).astype(_fp8 if FP8_UP else _bf16),
        "w2T": np.ascontiguousarray(g["w2"].T.reshape(8, 128, C)).astype(_bf16),
    }
    return out


def _build(n_rows=BPC, t_len=T):
    """Build + compile the per-core Bass program. Returns the finalized nc."""
    import concourse.bacc as bacc
    import concourse.bass as bass
    import concourse.mybir as mybir
    import concourse.tile as tile
    from concourse.masks import make_identity

    dt = mybir.dt
    AF = mybir.ActivationFunctionType
    OP = mybir.AluOpType
    DR = mybir.MatmulPerfMode.DoubleRow

    att_dt = dt.float8e4 if FP8_ATT else dt.bfloat16
    up_dt = dt.float8e4 if FP8_UP else dt.bfloat16

    nwin = t_len // WIN
    nc = bacc.Bacc("TRN2", target_bir_lowering=False, debug=False, num_devices=NCORES)

    x_d = nc.declare_dram_parameter("x", [n_rows, t_len, C], dt.float32, isOutput=False)
    widT_d = nc.declare_dram_parameter("widT", [128, 2, C], att_dt, isOutput=False)
    winfT_d = nc.declare_dram_parameter("winfT", [128, 2, C], att_dt, isOutput=False)
    wprojT_d = nc.declare_dram_parameter("wprojT", [2, 128, C], dt.bfloat16, isOutput=False)
    w1dr_d = nc.declare_dram_parameter("w1dr", [128, 2, 4 * C], up_dt, isOutput=False)
    w2T_d = nc.declare_dram_parameter("w2T", [8, 128, C], dt.bfloat16, isOutput=False)
    out_d = nc.declare_dram_parameter("out", [n_rows, t_len, C], dt.float32, isOutput=True)

    with tile.TileContext(nc) as tc, ExitStack() as ctx:
        singles = ctx.enter_context(tc.tile_pool(name="singles", bufs=1))
        hb_pool = ctx.enter_context(tc.tile_pool(name="hb", bufs=1))
        xin = ctx.enter_context(tc.tile_pool(name="xin", bufs=8))
        x1p = ctx.enter_context(tc.tile_pool(name="x1p", bufs=24))
        hnorm = ctx.enter_context(tc.tile_pool(name="hnorm", bufs=16))
        stats = ctx.enter_context(tc.tile_pool(name="stats", bufs=24))
        ybp = ctx.enter_context(tc.tile_pool(name="ybp", bufs=3))
        h2b = ctx.enter_context(tc.tile_pool(name="h2b", bufs=3))
        ffb = ctx.enter_context(tc.tile_pool(name="ffb", bufs=2))
        outp = ctx.enter_context(tc.tile_pool(name="outp", bufs=10))
        tp_ps = ctx.enter_context(tc.tile_pool(name="tp_ps", bufs=1, space="PSUM"))
        acc_ps = ctx.enter_context(tc.tile_pool(name="acc_ps", bufs=2, space="PSUM"))
        y_ps = ctx.enter_context(tc.tile_pool(name="y_ps", bufs=1, space="PSUM"))
        up_ps = ctx.enter_context(tc.tile_pool(name="up_ps", bufs=2, space="PSUM"))

        # ---- constants / weights in SBUF ----
        ident = singles.tile([128, 128], dt.bfloat16)
        make_identity(nc, ident)
        eps_t = singles.tile([128, 1], dt.float32)
        nc.vector.memset(eps_t, EPS)
        widT = singles.tile([128, 2, C], att_dt, tag="widT")
        nc.sync.dma_start(out=widT, in_=widT_d[:, :, :])
        winfT = singles.tile([128, 2, C], att_dt, tag="winfT")
        nc.sync.dma_start(out=winfT, in_=winfT_d[:, :, :])
        wprojT = []
        for kc in range(2):
            w = singles.tile([128, C], dt.bfloat16, tag=f"wprojT{kc}")
            nc.sync.dma_start(out=w, in_=wprojT_d[kc])
            wprojT.append(w)
        w1dr = singles.tile([128, 2, 4 * C], up_dt, tag="w1dr")
        nc.sync.dma_start(out=w1dr, in_=w1dr_d[:, :, :])
        w2T = []
        for fc in range(8):
            w = singles.tile([128, C], dt.bfloat16, tag=f"w2T{fc}")
            nc.sync.dma_start(out=w, in_=w2T_d[fc])
            w2T.append(w)

        def ln_stats(src_tile, mvW, k):
            st = stats.tile([128, 6], dt.float32, tag="st", name="st")
            nc.vector.bn_stats(st, src_tile)
            nc.vector.bn_aggr(mvW[:, 2 * k:2 * k + 2], st)

        def ln_batch_rsqrt(mvW):
            sdW = stats.tile([128, 4], dt.float32, tag="sd", name="sd")
            var_view = bass.AP(tensor=mvW.tensor, offset=mvW.offset + 1,
                               ap=[mvW.ap[0], [2, 4]])
            nc.scalar.activation(sdW, var_view, AF.Sqrt, bias=eps_t, scale=1.0)
            rsW = stats.tile([128, 4], dt.float32, tag="rs", name="rs")
            nc.vector.reciprocal(rsW, sdW)
            return rsW

        def ln_pair(src_tiles, mvW, rsW, kp, dst_tile, dst_col, dst_cstride,
                    copy_eng):
            """LN-apply + transpose two token subtiles (k = 2*kp, 2*kp+1) and
            copy the merged [128, 512] psum into the channel-major dst."""
            pt = tp_ps.tile([128, 4 * SUB], dt.float32, tag="tp", name="tp")
            for kk in range(2):
                k = 2 * kp + kk
                hn = hnorm.tile([128, C], dt.bfloat16, tag="hn", name="hn")
                nc.vector.tensor_scalar(
                    out=hn, in0=src_tiles[k], scalar1=mvW[:, 2 * k:2 * k + 1],
                    scalar2=rsW[:, k:k + 1], op0=OP.subtract, op1=OP.mult,
                )
                for c in range(2):
                    nc.tensor.matmul(
                        pt[:, (2 * kk + c) * SUB:(2 * kk + c + 1) * SUB],
                        hn[:, 128 * c:128 * (c + 1)], ident,
                        start=True, stop=True)
            src4 = bass.AP(tensor=pt.tensor, offset=pt.offset,
                           ap=[pt.ap[0], [SUB, 2], [2 * SUB, 2], [1, SUB]])
            dst4 = bass.AP(tensor=dst_tile.tensor,
                           offset=dst_tile.offset + dst_col,
                           ap=[dst_tile.ap[0], [dst_cstride, 2], [SUB, 2], [1, SUB]])
            if copy_eng == "s":
                nc.scalar.copy(out=dst4, in_=src4)
            else:
                nc.vector.tensor_copy(out=dst4, in_=src4)

        state = {}

        def x_load(slot, r, w):
            """One DMA (f32->bf16 cast) for window w's 4 x subtiles + LN1 stats.
            SBUF layout [128, 4, C]: token t0w + 128*k + p at [p, k, :]."""
            t0w = w * WIN
            xw = xin.tile([128, 4, C], dt.bfloat16, tag="x", name="x")
            src = x_d[r, t0w:t0w + WIN, :].rearrange("(k p) c -> p k c", p=128)
            nc.gpsimd.dma_start(out=xw, in_=src)
            x_tiles = [xw[:, k, :] for k in range(4)]
            mv1 = stats.tile([128, 8], dt.float32, tag="mv1", name="mv1")
            for k in range(4):
                ln_stats(x_tiles[k], mv1, k)
            state[("x", slot, w)] = x_tiles
            state[("mv1", slot, w)] = mv1

        def ln1_piece(slot, hB, w, kp):
            """LN1-apply + transpose of k-pair kp of window w into hB."""
            if kp == 0:
                state[("rs1", slot, w)] = ln_batch_rsqrt(state[("mv1", slot, w)])
            ln_pair(state[("x", slot, w)], state[("mv1", slot, w)],
                    state[("rs1", slot, w)], kp, hB,
                    PADW + w * WIN + kp * 2 * SUB, PADW + t_len, EG["hb_copy"])

        def stage1_phase(slot, hB, r, w):
            col0 = PADW + w * WIN
            yb = ybp.tile([128, 2, WIN], dt.bfloat16, tag=f"yb{slot}",
                          name=f"yb{slot}")
            state[("yb", slot)] = yb
            for mc in range(2):
                yp = y_ps.tile([128, WIN], dt.float32, tag="y", name="y")
                if FP8_ATT:
                    # id: full 128-partition dst -> DoubleRow ok.
                    nc.tensor.matmul(yp, widT[:, :, 128 * mc:128 * (mc + 1)],
                                     hB[:, :, col0:col0 + WIN],
                                     start=True, stop=False, perf_mode=DR)
                    # band 0 dst is partition base 0 -> DoubleRow ok
                    s = 2 * mc + 1
                    nc.tensor.matmul(
                        yp[0:64, :], winfT[:, :, 128 * mc:128 * mc + 64],
                        hB[:, :, col0 - s:col0 - s + WIN],
                        start=False, stop=False, perf_mode=DR)
                    # band 1 dst at partition 64: DR invalid -> normal fp8
                    s = 2 * mc + 2
                    for kc in range(2):
                        nc.tensor.matmul(
                            yp[64:128, :],
                            winfT[:, kc, 128 * mc + 64:128 * mc + 128],
                            hB[:, kc, col0 - s:col0 - s + WIN],
                            start=False, stop=(kc == 1))
                else:
                    for kc in range(2):
                        nc.tensor.matmul(yp, widT[:, kc, 128 * mc:128 * (mc + 1)],
                                         hB[:, kc, col0:col0 + WIN],
                                         start=(kc == 0), stop=False)
                    for b in range(2):
                        s = 2 * mc + b + 1
                        for kc in range(2):
                            nc.tensor.matmul(
                                yp[64 * b:64 * (b + 1), :],
                                winfT[:, kc, 128 * mc + 64 * b:128 * mc + 64 * (b + 1)],
                                hB[:, kc, col0 - s:col0 - s + WIN],
                                start=False, stop=(b == 1 and kc == 1))
                yscale = 1.0 / ATT_W_SCALE if FP8_ATT else 1.0
                nc.scalar.mul(yb[:, mc, :], yp, yscale)

        def stage2_phase(slot, r, w):
            x_tiles = state[("x", slot, w)]
            yb = state[("yb", slot)]
            x1_tiles = []
            mv2 = stats.tile([128, 8], dt.float32, tag="mv2", name="mv2")
            for k in range(4):
                ps = acc_ps.tile([128, C], dt.float32, tag="acc", name="sa")
                for kc in range(2):
                    nc.tensor.matmul(ps, yb[:, kc, k * SUB:(k + 1) * SUB],
                                     wprojT[kc], start=(kc == 0), stop=(kc == 1))
                x1t = x1p.tile([128, C], dt.bfloat16, tag="x1", name="x1")
                nc.vector.tensor_add(out=x1t, in0=x_tiles[k], in1=ps)
                x1_tiles.append(x1t)
                ln_stats(x1t, mv2, k)
            state[("x1", slot, w)] = x1_tiles
            state[("mv2", slot)] = mv2

        def dn_group(slot, r, w, k, fftiles, x1_tiles):
            t0 = w * WIN + k * SUB
            pd = acc_ps.tile([128, C], dt.float32, tag="acc", name="dn")
            for fc in range(8):
                nc.tensor.matmul(pd, fftiles[fc][:, k * SUB:(k + 1) * SUB],
                                 w2T[fc], start=(fc == 0), stop=(fc == 7))
            ot = outp.tile([128, C], dt.float32, tag="o", name="o")
            nc.vector.tensor_add(out=ot, in0=x1_tiles[k], in1=pd)
            nc.sync.dma_start(out=out_d[r, t0:t0 + SUB, :], in_=ot)

        def ln2_phase(slot, w):
            x1_tiles = state[("x1", slot, w)]
            mv2 = state[("mv2", slot)]
            rs2 = ln_batch_rsqrt(mv2)
            h2 = h2b.tile([128, 2, WIN], up_dt, tag=f"h2dr{slot}",
                          name=f"h2dr{slot}")
            for kp in range(2):
                ln_pair(x1_tiles, mv2, rs2, kp, h2, kp * 2 * SUB, WIN,
                        EG["h2_copy"])
            state[("h2", slot)] = h2

        def up_pairs(slot, r, w, fillers):
            """FFN-up for w; `fillers` (LN1 pieces of w+1, x loads of w+2)
            are emitted between up pairs to keep the PE stream dense."""
            h2 = state[("h2", slot)]
            fftiles = []
            fill = iter(fillers)
            for p in range(4):  # fc pairs
                pu = up_ps.tile([128, 8 * SUB], dt.float32, tag="up", name="up")
                for half in range(2):
                    fc = 2 * p + half
                    if FP8_UP:
                        nc.tensor.matmul(pu[:, half * WIN:(half + 1) * WIN],
                                         w1dr[:, :, 128 * fc:128 * (fc + 1)], h2,
                                         start=True, stop=True, perf_mode=DR)
                    else:
                        nc.tensor.matmul(pu[:, half * WIN:(half + 1) * WIN],
                                         w1dr[:, 0, 128 * fc:128 * (fc + 1)],
                                         h2[:, 0, :], start=True, stop=False)
                        nc.tensor.matmul(pu[:, half * WIN:(half + 1) * WIN],
                                         w1dr[:, 1, 128 * fc:128 * (fc + 1)],
                                         h2[:, 1, :], start=False, stop=True)
                fb = ffb.tile([128, 2, WIN], dt.bfloat16, tag=f"ffp{slot}{p}",
                              name=f"ffp{slot}{p}")
                # b1_eff == 0 (asserted in _prep): relu(pu/16)
                nc.scalar.activation(fb, pu, AF.Relu, bias=0.0,
                                     scale=1.0 / UP_W_SCALE)
                fftiles.append(fb)
                f = next(fill, None)
                if f is not None:
                    f()
            for f in fill:
                f()
            ffv = []
            for p in range(4):
                for half in range(2):
                    ffv.append(fftiles[p][:, half, :])
            state[("ff", slot)] = (r, w, ffv, state[("x1", slot, w)])

        def dn_piece(slot, k):
            pv = state[("ff", slot)]
            if pv is not None:
                dn_group(slot, pv[0], pv[1], k, pv[2], pv[3])

        # ---- main pipeline ----
        # Fine-grained per-window emission: the two slots' phases are woven
        # so PE-dense blocks (stage1, dn, up pairs) cover the other slot's
        # LN chains and psum->sbuf copy latencies.
        nslots = min(2, n_rows)
        for rp in range(0, n_rows, nslots):
            hBs = []
            for slot in range(nslots):
                hB = hb_pool.tile([128, 2, PADW + t_len], att_dt,
                                  tag=f"hb{slot}", name=f"hb{slot}")
                nc.gpsimd.memset(hB[:, :, 0:PADW], 0.0)
                hBs.append(hB)
            for slot in range(nslots):
                state[("ff", slot)] = None
                r = rp + slot
                x_load(slot, r, 0)
                x_load(slot, r, 1)
                ln1_piece(slot, hBs[slot], 0, 0)
                ln1_piece(slot, hBs[slot], 0, 1)
            s0, s1 = 0, 1 % nslots
            for w in range(nwin):
                r0, r1 = rp + s0, rp + s1
                stage1_phase(s0, hBs[s0], r0, w)
                dn_piece(s0, 0)
                dn_piece(s0, 1)
                stage2_phase(s0, r0, w)
                dn_piece(s0, 2)
                if nslots > 1:
                    stage1_phase(s1, hBs[s1], r1, w)
                ln2_phase(s0, w)
                if nslots > 1:
                    stage2_phase(s1, r1, w)
                fillers = [lambda slot=s0: dn_piece(slot, 3)]
                if w + 1 < nwin:
                    for kp in range(2):
                        fillers.append(lambda slot=s0, w=w, kp=kp:
                                       ln1_piece(slot, hBs[slot], w + 1, kp))
                if w + 2 < nwin:
                    fillers.append(lambda slot=s0, r=r0, w=w:
                                   x_load(slot, r, w + 2))
                up_pairs(s0, r0, w, fillers)
                if nslots > 1:
                    dn_piece(s1, 0)
                    dn_piece(s1, 1)
                    ln2_phase(s1, w)
                    dn_piece(s1, 2)
                    fillers = [lambda slot=s1: dn_piece(slot, 3)]
                    if w + 1 < nwin:
                        for kp in range(2):
                            fillers.append(lambda slot=s1, w=w, kp=kp:
                                           ln1_piece(slot, hBs[slot], w + 1, kp))
                    if w + 2 < nwin:
                        fillers.append(lambda slot=s1, r=r1, w=w:
                                       x_load(slot, r, w + 2))
                    up_pairs(s1, r1, w, fillers)
            for slot in range(nslots):
                r_, w_, ff_, x1_ = state[("ff", slot)]
                for k in range(4):
                    dn_group(slot, r_, w_, k, ff_, x1_)
                state[("ff", slot)] = None

    nc.compile()
    return nc


_CACHE = {}


def _get_nc():
    if "nc" not in _CACHE:
        _CACHE["nc"] = _build()
    return _CACHE["nc"]


def _run(inputs, trace_dir=None):
    from concourse.bass_utils import run_bass_kernel_spmd
    from concourse import bass2jax

    x = np.asarray(inputs["x"], dtype=np.float32)
    w = _prep(inputs)
    nc = _get_nc()

    in_maps = []
    for core in range(NCORES):
        m = dict(w)
        m["x"] = np.ascontiguousarray(x[core * BPC:(core + 1) * BPC])
        in_maps.append(m)

    if trace_dir is None:
        res = run_bass_kernel_spmd(nc, in_maps, list(range(NCORES)))
        results, exec_ns = res.results, None
    else:
        import ctypes
        from contextlib import contextmanager

        lib = ctypes.CDLL("/opt/axon/libaxon_pjrt.so")
        lib.axon_start_nrt_profile.argtypes = [
            ctypes.POINTER(ctypes.c_int64), ctypes.c_size_t]
        lib.axon_start_nrt_profile.restype = ctypes.c_int64
        lib.axon_stop_nrt_profile.argtypes = [ctypes.c_char_p]
        lib.axon_stop_nrt_profile.restype = ctypes.c_int64

        @contextmanager
        def hook(output_dir, device_ids):
            import jax
            jax.devices()
            ids = (ctypes.c_int64 * len(device_ids))(*device_ids)
            rc = lib.axon_start_nrt_profile(ids, len(device_ids))
            if rc != 0:
                raise RuntimeError(f"axon_start_nrt_profile rc={rc}")
            try:
                yield
            finally:
                n = lib.axon_stop_nrt_profile(str(output_dir).encode())
                print(f"profile: {n} file(s) written to {output_dir}")

        os.makedirs(trace_dir, exist_ok=True)
        with hook(trace_dir, [0]):
            results = bass2jax.run_bass_via_pjrt(nc, in_maps, n_cores=NCORES)
        exec_ns = None  # caller post-processes the NTFFs

    out = np.concatenate([np.asarray(results[i]["out"]) for i in range(NCORES)], axis=0)
    return out, exec_ns


def kernel(**inputs):
    out, _ = _run(inputs)
    return out
